# revision 21
# baseline (speedup 1.0000x reference)
"""Trainium2 Bass kernel for CentroidLayer inference.

reference math:
    _, V = eigh(C)                              # [NC, N_CH, P, P]
    diag[b,c,n,i] = sum_{j,k} V[c,n,j,i] * X[b,n,j,k] * V[c,n,k,i]

Strategy:
  * eigh(C) on host (eigenvector sign ambiguity cancels in the bilinear
    form, so any correct eigh matches the reference up to fp rounding).
  * Shard the 8 channels (N_CH) across the 8 NeuronCores — each core
    handles one channel end-to-end; inputs/outputs per core are 1/8 of
    the total, and the centroid data for one channel is tiny (128 KB).
  * Per core, rewrite the contraction as one big matmul:
        out[b, (c,i)] = sum_{jk} X[b, (j,k)] * W[(j,k), (c,i)]
        W[(j,k), (c,i)] = V[c,j,i] * V[c,k,i]
    W ([1024, 1024], 4 MB) is built ON DEVICE from V (128 KB) to keep
    HBM traffic minimal:
        Urep[(jj,k), ci] = U[k, ci]      (U tiled 4x on partitions; DMA'd)
        Ubc [(j,k),  ci] = U[j, ci]      (PE: selection-matrix matmuls)
        W = Ubc * Urep                   (DVE elementwise)
    Main matmuls run in float32r (4x faster than float32 on the PE).
"""

import os
import sys

import numpy as np

B, NC, N_CH, P = 256, 32, 8, 32
CI = NC * P          # 1024 (c,i) pairs
JK = P * P           # 1024 (j,k) pairs
NKC = JK // 128      # 8 contraction chunks of 128
NBH = B // 128       # 2 batch halves of 128

# dtype knobs for the PE (empirically tuned; float32r is the fast fp32 path)
MAIN_F32R = os.environ.get("KERNEL_MAIN_F32R", "1") == "1"
BSEL_F32R = os.environ.get("KERNEL_BSEL_F32R", "1") == "1"

_PROGRAM = None


def _import_concourse():
    try:
        import concourse  # noqa: F401
    except ImportError:
        for p in ("/opt/trn_rl_repo", os.path.expanduser("~/trn_rl_repo")):
            if os.path.isdir(p):
                sys.path.insert(0, p)
                break
        import concourse  # noqa: F401
    _ensure_axon_hooks()


def _ensure_axon_hooks():
    """This image's `antenv` lacks `axon_hooks`; concourse imports it when
    trace=True. Provide the module + register the ctypes NTFF hook so
    profiling works (best-effort; everything still runs without it)."""
    try:
        import antenv.axon_hooks  # noqa: F401

        return
    except ImportError:
        pass
    try:
        import types

        import antenv

        mod = types.ModuleType("antenv.axon_hooks")
        holder = {"hook": None}
        mod.set_axon_ntff_profile_hook = lambda h: holder.__setitem__("hook", h)
        mod.get_axon_ntff_profile_hook = lambda: holder["hook"]
        sys.modules["antenv.axon_hooks"] = mod
        antenv.axon_hooks = mod
        boot_dir = "/root/.axon_site/trn_agent_boot"
        so_path = "/opt/axon/libaxon_pjrt.so"
        if os.path.isdir(boot_dir) and os.path.exists(so_path):
            if boot_dir not in sys.path:
                sys.path.insert(0, boot_dir)
            from trn_boot import _ntff_profile_via_ctypes

            holder["hook"] = _ntff_profile_via_ctypes(so_path)
    except Exception:
        pass


def _build_program():
    """Bass program for ONE core (one channel). SPMD across 8 cores."""
    import concourse.bacc as bacc
    import concourse.mybir as mybir
    from concourse.tile import TileContext

    f32 = mybir.dt.float32
    f32r = mybir.dt.float32r
    # fp32r = fp32 rounded to an 11-bit mantissa (low 12 bits zero), runs the
    # PE at 4x the fp32 rate. The BIR verifier requires every matmul operand's
    # producer to emit float32r-typed output, so the dtype is threaded through
    # DRAM params and SBUF tiles; host pre-rounds the values to the f32r grid.
    main_dt = f32r if MAIN_F32R else f32
    bsel_dt = f32r if BSEL_F32R else f32

    bf16 = mybir.dt.bfloat16

    nc = bacc.Bacc()
    xt_d = nc.declare_dram_parameter(
        "xt", [128, NBH * NKC * 128], main_dt, isOutput=False
    )
    urep_d = nc.declare_dram_parameter("urep", [128, CI], bsel_dt, isOutput=False)
    out_d = nc.declare_dram_parameter("out", [B, CI], f32, isOutput=True)

    with TileContext(nc) as tc:
        with (
            tc.tile_pool(name="const", bufs=1) as const_pool,
            tc.tile_pool(name="w", bufs=NKC) as w_pool,
            tc.tile_pool(name="ob", bufs=2) as o_pool,
            tc.tile_pool(name="ubc", bufs=2, space="PSUM") as ubc_pool,
            tc.tile_pool(name="acc", bufs=4, space="PSUM") as acc_pool,
        ):
            # --- PE warmup: ~4us of dummy matmuls during the DMA wait trips
            # the HAM clock gate to 8/8 so the real matmuls run at 2.4 GHz.
            warm = const_pool.tile([128, 512], bf16, name="warm")
            nc.gpsimd.memset(warm[:], 0.0)
            warm_ps = acc_pool.tile([128, 512], f32, tag="acc", name="warm_ps")
            for i in range(10):
                nc.tensor.matmul(
                    warm_ps[:], lhsT=warm[:, 0:128], rhs=warm[:], start=True, stop=True
                )

            urep = const_pool.tile([128, CI], bsel_dt, name="urep")
            nc.sync.dma_start(urep[:], urep_d[:])
            xt = const_pool.tile([128, NBH * NKC * 128], main_dt, name="xt")
            for bh in range(NBH):
                s = bh * NKC * 128
                nc.sync.dma_start(xt[:, s : s + NKC * 128], xt_d[:, s : s + NKC * 128])

            # --- synthesize bsel on device (gpsimd iota + DVE compare):
            # bsel[32*(kc%4)+j, kc*128+p] = 1 iff j == 4*kc + p//32
            # row target per column: F(col) = 36*kc + p//32 - 128*(kc >= 4)
            tcol = const_pool.tile([128, NKC * 128], f32, name="tcol")
            rrow = const_pool.tile([128, 1], f32, name="rrow")
            nc.gpsimd.iota(
                rrow[:], [[0, 1]], base=0, channel_multiplier=1,
                allow_small_or_imprecise_dtypes=True,
            )
            nc.gpsimd.iota(
                tcol[:, 0:512], [[36, 4], [1, 4], [0, 32]], base=0,
                channel_multiplier=0, allow_small_or_imprecise_dtypes=True,
            )
            nc.gpsimd.iota(
                tcol[:, 512:1024], [[36, 4], [1, 4], [0, 32]], base=16,
                channel_multiplier=0, allow_small_or_imprecise_dtypes=True,
            )
            bsel = const_pool.tile([128, NKC * 128], bsel_dt, name="bsel")
            nc.vector.tensor_tensor(
                bsel[:], tcol[:], rrow[:].to_broadcast((128, NKC * 128)),
                op=mybir.AluOpType.is_equal,
            )

            # --- build W chunks: W[kc][(j,k) in chunk, (c,i)] ---
            wts = []
            for kc in range(NKC):
                r = kc % 4
                ubc = ubc_pool.tile([128, CI], f32, tag="ubc", name=f"ubc_{kc}")
                wt = w_pool.tile([128, CI], main_dt, tag="wt", name=f"wt_{kc}")
                for nh in range(CI // 512):
                    sl = slice(nh * 512, (nh + 1) * 512)
                    nc.tensor.matmul(
                        ubc[:, sl],
                        lhsT=bsel[32 * r : 32 * r + 32, kc * 128 : (kc + 1) * 128],
                        rhs=urep[32 * r : 32 * r + 32, sl],
                        start=True,
                        stop=True,
                        tile_position=(32 * r, 0),
                    )
                    nc.vector.tensor_mul(
                        wt[:, sl], ubc[:, sl], urep[:, sl].bitcast(f32)
                    )
                wts.append(wt)

            # --- main contraction: out[b, ci] = sum_kc xt_kc^T @ W_kc ---
            for bh in range(NBH):
                accs = [
                    acc_pool.tile([128, 512], f32, tag="acc", name=f"acc_{bh}_{i}")
                    for i in range(2)
                ]
                for kc in range(NKC):
                    lhs = xt[
                        :, bh * NKC * 128 + kc * 128 : bh * NKC * 128 + (kc + 1) * 128
                    ]
                    for nh in range(2):
                        nc.tensor.matmul(
                            accs[nh][:],
                            lhsT=lhs,
                            rhs=wts[kc][:, nh * 512 : (nh + 1) * 512],
                            start=(kc == 0),
                            stop=(kc == NKC - 1),
                        )
                ob = o_pool.tile([128, CI], f32, tag="ob", name=f"ob_{bh}")
                for nh in range(2):
                    nc.scalar.copy(ob[:, nh * 512 : (nh + 1) * 512], accs[nh][:])
                nc.sync.dma_start(out_d[bh * 128 : (bh + 1) * 128, :], ob[:])

    nc.finalize()
    return nc


def _get_program():
    global _PROGRAM
    if _PROGRAM is None:
        _import_concourse()
        _PROGRAM = _build_program()
    return _PROGRAM


def _eigvecs(C):
    # jax CPU eigh reproduces the reference's eigenvectors bit-for-bit;
    # a from-scratch f64 eigh would differ by the reference's own f32 eigh
    # error (~3e-4 in the output) on near-degenerate eigenpairs.
    try:
        import jax
        import jax.numpy as jnp

        with jax.default_device(jax.devices("cpu")[0]):
            _, V = jnp.linalg.eigh(jnp.asarray(C, dtype=jnp.float32))
            return np.asarray(V)
    except Exception:
        _, V = np.linalg.eigh(C.astype(np.float64))
        return V.astype(np.float32)


def _round_fp32r(a):
    """Round to the fp32r grid (11-bit mantissa, RNE), matching the PE's
    fp32_to_fp32r downconversion. Idempotent; exact on 0/1."""
    u = np.ascontiguousarray(a, dtype=np.float32).view(np.uint32)
    lsb = (u >> 12) & np.uint32(1)
    r = u + np.uint32(0x7FF) + lsb
    return (r & np.uint32(0xFFFFF000)).view(np.float32)


def _host_prep(X, C):
    """Host-side: eigh + per-core input layouts."""
    X = np.ascontiguousarray(np.asarray(X, dtype=np.float32))
    C = np.asarray(C, dtype=np.float32)

    V = _eigvecs(C)  # [NC, N_CH, P(j), P(i)]
    if MAIN_F32R:
        X = _round_fp32r(X)
    if BSEL_F32R:
        V = _round_fp32r(V)

    # U[n][k, c*P+i] = V[c, n, k, i]
    U = V.transpose(1, 2, 0, 3).reshape(N_CH, P, CI)
    urep = np.ascontiguousarray(np.tile(U, (1, 4, 1)))  # [n, 128, CI]

    # xt[n][p, bh*1024 + kc*128 + bb] = X[bh*128+bb, n, j, k], jk = kc*128+p
    Xt = X.transpose(1, 2, 3, 0).reshape(N_CH, NKC, 128, NBH, 128)
    xt = np.ascontiguousarray(
        Xt.transpose(0, 2, 3, 1, 4).reshape(N_CH, 128, NBH * NKC * 128)
    )

    return xt, urep


def _reassemble(outs):
    # outs: list of 8 arrays [B, CI]; diag[b, c, n, i] = outs[n][b, c*P+i]
    full = np.stack(outs, axis=0).reshape(N_CH, B, NC, P)
    return np.ascontiguousarray(full.transpose(1, 2, 0, 3))


LAST_RESULTS = None  # BassKernelResults from the most recent device run


def kernel(X, C, idx=None, **_unused):
    global LAST_RESULTS
    _import_concourse()

    xt, urep = _host_prep(X, C)
    nc = _get_program()
    in_maps = [{"xt": xt[n], "urep": urep[n]} for n in range(N_CH)]

    if os.environ.get("KERNEL_SIM", "0") == "1":
        from concourse import bass_interp

        sim = bass_interp.MultiCoreSim(nc, N_CH)
        for n in range(N_CH):
            for name, arr in in_maps[n].items():
                sim.cores[n].tensor(name)[:] = arr
        sim.simulate()
        outs = [np.array(sim.cores[n].mem_tensor("out")) for n in range(N_CH)]
    else:
        from concourse import bass_utils

        res = bass_utils.run_bass_kernel_spmd(
            nc,
            in_maps,
            list(range(N_CH)),
            trace=os.environ.get("KERNEL_TRACE", "0") == "1",
        )
        LAST_RESULTS = res
        outs = [res.results[n]["out"] for n in range(N_CH)]

    return _reassemble(outs)


# revision 24
# speedup vs baseline: 1.3541x; 1.3541x over previous
"""Trainium2 Bass kernel for CentroidLayer inference.

reference math:
    _, V = eigh(C)                              # [NC, N_CH, P, P]
    diag[b,c,n,i] = sum_{j,k} V[c,n,j,i] * X[b,n,j,k] * V[c,n,k,i]

Strategy:
  * eigh(C) on host (eigenvector sign ambiguity cancels in the bilinear
    form, so any correct eigh matches the reference up to fp rounding).
  * Shard the 8 channels (N_CH) across the 8 NeuronCores — each core
    handles one channel end-to-end; inputs/outputs per core are 1/8 of
    the total, and the centroid data for one channel is tiny (128 KB).
  * Per core, rewrite the contraction as one big matmul:
        out[b, (c,i)] = sum_{jk} X[b, (j,k)] * W[(j,k), (c,i)]
        W[(j,k), (c,i)] = V[c,j,i] * V[c,k,i]
    W ([1024, 1024], 4 MB) is built ON DEVICE from V (128 KB) to keep
    HBM traffic minimal:
        Urep[(jj,k), ci] = U[k, ci]      (U tiled 4x on partitions; DMA'd)
        Ubc [(j,k),  ci] = U[j, ci]      (PE: selection-matrix matmuls)
        W = Ubc * Urep                   (DVE elementwise)
    Main matmuls run in float32r (4x faster than float32 on the PE).
"""

import os
import sys

import numpy as np

B, NC, N_CH, P = 256, 32, 8, 32
CI = NC * P          # 1024 (c,i) pairs
JK = P * P           # 1024 (j,k) pairs
NKC = JK // 128      # 8 contraction chunks of 128
NBH = B // 128       # 2 batch halves of 128

# dtype knobs for the PE (empirically tuned; float32r is the fast fp32 path)
MAIN_F32R = os.environ.get("KERNEL_MAIN_F32R", "1") == "1"
BSEL_F32R = os.environ.get("KERNEL_BSEL_F32R", "1") == "1"

_PROGRAM = None


def _import_concourse():
    try:
        import concourse  # noqa: F401
    except ImportError:
        for p in ("/opt/trn_rl_repo", os.path.expanduser("~/trn_rl_repo")):
            if os.path.isdir(p):
                sys.path.insert(0, p)
                break
        import concourse  # noqa: F401
    _ensure_axon_hooks()


def _ensure_axon_hooks():
    """This image's `antenv` lacks `axon_hooks`; concourse imports it when
    trace=True. Provide the module + register the ctypes NTFF hook so
    profiling works (best-effort; everything still runs without it)."""
    try:
        import antenv.axon_hooks  # noqa: F401

        return
    except ImportError:
        pass
    try:
        import types

        import antenv

        mod = types.ModuleType("antenv.axon_hooks")
        holder = {"hook": None}
        mod.set_axon_ntff_profile_hook = lambda h: holder.__setitem__("hook", h)
        mod.get_axon_ntff_profile_hook = lambda: holder["hook"]
        sys.modules["antenv.axon_hooks"] = mod
        antenv.axon_hooks = mod
        boot_dir = "/root/.axon_site/trn_agent_boot"
        so_path = "/opt/axon/libaxon_pjrt.so"
        if os.path.isdir(boot_dir) and os.path.exists(so_path):
            if boot_dir not in sys.path:
                sys.path.insert(0, boot_dir)
            from trn_boot import _ntff_profile_via_ctypes

            holder["hook"] = _ntff_profile_via_ctypes(so_path)
    except Exception:
        pass


def _build_program():
    """Bass program for ONE core (one channel). SPMD across 8 cores."""
    import concourse.bacc as bacc
    import concourse.mybir as mybir
    from concourse.tile import TileContext

    f32 = mybir.dt.float32
    f32r = mybir.dt.float32r
    # fp32r = fp32 rounded to an 11-bit mantissa (low 12 bits zero), runs the
    # PE at 4x the fp32 rate. The BIR verifier requires every matmul operand's
    # producer to emit float32r-typed output, so the dtype is threaded through
    # DRAM params and SBUF tiles; host pre-rounds the values to the f32r grid.
    main_dt = f32r if MAIN_F32R else f32
    bsel_dt = f32r if BSEL_F32R else f32

    bf16 = mybir.dt.bfloat16

    nc = bacc.Bacc()
    xt_d = nc.declare_dram_parameter(
        "xt", [128, NBH * NKC * 128], main_dt, isOutput=False
    )
    urep_d = nc.declare_dram_parameter("urep", [128, CI], bsel_dt, isOutput=False)
    out_d = nc.declare_dram_parameter("out", [B, CI], f32, isOutput=True)

    with TileContext(nc) as tc:
        with (
            tc.tile_pool(name="const", bufs=1) as const_pool,
            tc.tile_pool(name="w", bufs=NKC) as w_pool,
            tc.tile_pool(name="ob", bufs=2) as o_pool,
            tc.tile_pool(name="ubc", bufs=4, space="PSUM") as ubc_pool,
            tc.tile_pool(name="acc", bufs=4, space="PSUM") as acc_pool,
        ):
            # --- PE warmup: ~5us of dummy matmuls during the DMA wait trips
            # the HAM clock gate to 8/8 so the real matmuls run at 2.4 GHz.
            # Data must NOT be all-zero/all-one (zero-skip would idle the PE).
            warm = const_pool.tile([128, 512], bf16, name="warm")
            nc.gpsimd.iota(
                warm[:], [[1, 512]], base=0, channel_multiplier=3,
                allow_small_or_imprecise_dtypes=True,
            )
            warm_ps = acc_pool.tile([128, 512], f32, tag="acc", name="warm_ps")
            for i in range(14):
                nc.tensor.matmul(
                    warm_ps[:], lhsT=warm[:, 0:128], rhs=warm[:], start=True, stop=True
                )

            urep = const_pool.tile([128, CI], bsel_dt, name="urep")
            nc.sync.dma_start(urep[:], urep_d[:])
            xt = const_pool.tile([128, NBH * NKC * 128], main_dt, name="xt")
            for bh in range(NBH):
                s = bh * NKC * 128
                nc.sync.dma_start(xt[:, s : s + NKC * 128], xt_d[:, s : s + NKC * 128])

            # --- synthesize bsel on device (gpsimd iota + DVE compare):
            # bsel[32*(kc%4)+j, kc*128+p] = 1 iff j == 4*kc + p//32
            # row target per column: F(col) = 36*kc + p//32 - 128*(kc >= 4)
            tcol = const_pool.tile([128, NKC * 128], f32, name="tcol")
            rrow = const_pool.tile([128, 1], f32, name="rrow")
            nc.gpsimd.iota(
                rrow[:], [[0, 1]], base=0, channel_multiplier=1,
                allow_small_or_imprecise_dtypes=True,
            )
            nc.gpsimd.iota(
                tcol[:, 0:512], [[36, 4], [1, 4], [0, 32]], base=0,
                channel_multiplier=0, allow_small_or_imprecise_dtypes=True,
            )
            nc.gpsimd.iota(
                tcol[:, 512:1024], [[36, 4], [1, 4], [0, 32]], base=16,
                channel_multiplier=0, allow_small_or_imprecise_dtypes=True,
            )
            bsel = const_pool.tile([128, NKC * 128], bsel_dt, name="bsel")
            nc.vector.tensor_tensor(
                bsel[:], tcol[:], rrow[:].to_broadcast((128, NKC * 128)),
                op=mybir.AluOpType.is_equal,
            )

            # --- build W chunks: W[kc][(j,k) in chunk, (c,i)] ---
            wts = []
            for kc in range(NKC):
                r = kc % 4
                wt = w_pool.tile([128, CI], main_dt, tag="wt", name=f"wt_{kc}")
                for nh in range(CI // 512):
                    sl = slice(nh * 512, (nh + 1) * 512)
                    ubc = ubc_pool.tile(
                        [128, 512], f32, tag="ubc", name=f"ubc_{kc}_{nh}"
                    )
                    nc.tensor.matmul(
                        ubc[:],
                        lhsT=bsel[32 * r : 32 * r + 32, kc * 128 : (kc + 1) * 128],
                        rhs=urep[32 * r : 32 * r + 32, sl],
                        start=True,
                        stop=True,
                        tile_position=(32 * r, 0),
                    )
                    nc.vector.tensor_mul(wt[:, sl], ubc[:], urep[:, sl].bitcast(f32))
                wts.append(wt)

            # --- main contraction: out[b, ci] = sum_kc xt_kc^T @ W_kc ---
            for bh in range(NBH):
                accs = [
                    acc_pool.tile([128, 512], f32, tag="acc", name=f"acc_{bh}_{i}")
                    for i in range(2)
                ]
                for kc in range(NKC):
                    lhs = xt[
                        :, bh * NKC * 128 + kc * 128 : bh * NKC * 128 + (kc + 1) * 128
                    ]
                    for nh in range(2):
                        nc.tensor.matmul(
                            accs[nh][:],
                            lhsT=lhs,
                            rhs=wts[kc][:, nh * 512 : (nh + 1) * 512],
                            start=(kc == 0),
                            stop=(kc == NKC - 1),
                        )
                ob = o_pool.tile([128, CI], f32, tag="ob", name=f"ob_{bh}")
                for nh in range(2):
                    nc.scalar.copy(ob[:, nh * 512 : (nh + 1) * 512], accs[nh][:])
                nc.sync.dma_start(out_d[bh * 128 : (bh + 1) * 128, :], ob[:])

    nc.finalize()
    return nc


def _get_program():
    global _PROGRAM
    if _PROGRAM is None:
        _import_concourse()
        _PROGRAM = _build_program()
    return _PROGRAM


def _eigvecs(C):
    # jax CPU eigh reproduces the reference's eigenvectors bit-for-bit;
    # a from-scratch f64 eigh would differ by the reference's own f32 eigh
    # error (~3e-4 in the output) on near-degenerate eigenpairs.
    try:
        import jax
        import jax.numpy as jnp

        with jax.default_device(jax.devices("cpu")[0]):
            _, V = jnp.linalg.eigh(jnp.asarray(C, dtype=jnp.float32))
            return np.asarray(V)
    except Exception:
        _, V = np.linalg.eigh(C.astype(np.float64))
        return V.astype(np.float32)


def _round_fp32r(a):
    """Round to the fp32r grid (11-bit mantissa, RNE), matching the PE's
    fp32_to_fp32r downconversion. Idempotent; exact on 0/1."""
    u = np.ascontiguousarray(a, dtype=np.float32).view(np.uint32)
    lsb = (u >> 12) & np.uint32(1)
    r = u + np.uint32(0x7FF) + lsb
    return (r & np.uint32(0xFFFFF000)).view(np.float32)


def _host_prep(X, C):
    """Host-side: eigh + per-core input layouts."""
    X = np.ascontiguousarray(np.asarray(X, dtype=np.float32))
    C = np.asarray(C, dtype=np.float32)

    V = _eigvecs(C)  # [NC, N_CH, P(j), P(i)]
    if MAIN_F32R:
        X = _round_fp32r(X)
    if BSEL_F32R:
        V = _round_fp32r(V)

    # U[n][k, c*P+i] = V[c, n, k, i]
    U = V.transpose(1, 2, 0, 3).reshape(N_CH, P, CI)
    urep = np.ascontiguousarray(np.tile(U, (1, 4, 1)))  # [n, 128, CI]

    # xt[n][p, bh*1024 + kc*128 + bb] = X[bh*128+bb, n, j, k], jk = kc*128+p
    Xt = X.transpose(1, 2, 3, 0).reshape(N_CH, NKC, 128, NBH, 128)
    xt = np.ascontiguousarray(
        Xt.transpose(0, 2, 3, 1, 4).reshape(N_CH, 128, NBH * NKC * 128)
    )

    return xt, urep


def _reassemble(outs):
    # outs: list of 8 arrays [B, CI]; diag[b, c, n, i] = outs[n][b, c*P+i]
    full = np.stack(outs, axis=0).reshape(N_CH, B, NC, P)
    return np.ascontiguousarray(full.transpose(1, 2, 0, 3))


LAST_RESULTS = None  # BassKernelResults from the most recent device run


def kernel(X, C, idx=None, **_unused):
    global LAST_RESULTS
    _import_concourse()

    xt, urep = _host_prep(X, C)
    nc = _get_program()
    in_maps = [{"xt": xt[n], "urep": urep[n]} for n in range(N_CH)]

    if os.environ.get("KERNEL_SIM", "0") == "1":
        from concourse import bass_interp

        sim = bass_interp.MultiCoreSim(nc, N_CH)
        for n in range(N_CH):
            for name, arr in in_maps[n].items():
                sim.cores[n].tensor(name)[:] = arr
        sim.simulate()
        outs = [np.array(sim.cores[n].mem_tensor("out")) for n in range(N_CH)]
    else:
        from concourse import bass_utils

        res = bass_utils.run_bass_kernel_spmd(
            nc,
            in_maps,
            list(range(N_CH)),
            trace=os.environ.get("KERNEL_TRACE", "0") == "1",
        )
        LAST_RESULTS = res
        outs = [res.results[n]["out"] for n in range(N_CH)]

    return _reassemble(outs)
